# revision 14
# baseline (speedup 1.0000x reference)
"""Trainium2 distributed kernel for nn_ActionThenNodePolicy.

Data-parallel over graphs: 8 cores x 256 graphs (16384 nodes) each.
All segment reductions are device-local (64-node contiguous segments).

Math (per graph g, 64 nodes):
  l = V @ Wcat.T  with Wcat = [W_node; W_agn; W_nga; 0.5*(W_q[:32]+W_q[32:])]
  p_n        = exp(l_node) / seg_sum(exp(l_node))
  p_a_given_n= exp(l_agn)*m1 / row_sum(exp(l_agn)*m1)
  p_a        = seg_sum(p_n * p_a_given_n)
  p_n__a     = exp(l_nga)*m2 / seg_sum(exp(l_nga)*m2)
  t          = seg_sum(p_n__a * q) ;  h = seg_sum(p_n__a*log(p_n__a+eps))
(masking by multiply-with-{0,1} is exact vs the reference's -1e9 + exp underflow;
 max-subtraction is skipped since |logits| < ~10 so exp cannot overflow)

Device layout: nodes on partitions (128/tile), all elementwise work natural.
V is host-pre-transposed to [D, N] bf16 so the matmul lhsT (contraction d on
partitions) DMAs contiguously.  Segment sums over the 64-node halves of each
partition tile are tiny indicator matmuls; partition-broadcast back is a K=2
matmul.  Final [G]-sized assembly (logprob gather, entropy/value dots) on host.
"""

import numpy as np
import ml_dtypes

import concourse.bass as bass
from concourse import bacc
from concourse import mybir
from concourse.tile import TileContext
from concourse.bass_utils import run_bass_kernel_spmd

N_GRAPHS = 2048
NPG = 64            # nodes per graph
N = N_GRAPHS * NPG  # 131072
D = 256
A = 32
HEADS = 2
NEG = -1e9
EPS = 1e-20

CORES = 8
NPC = N // CORES          # 16384 nodes per core
GPC = N_GRAPHS // CORES   # 256 graphs per core
P = 128                   # partitions / nodes per tile
TILES = NPC // P          # 128 tiles per core
TPB = 8                   # tiles per batch
NB = TILES // TPB         # 16 batches
F = 97                    # fused logit columns: 1 node + 32 agn + 32 nga + 32 q

_BF16 = mybir.dt.bfloat16
_F32 = mybir.dt.float32
_F32R = mybir.dt.float32r

_CACHE = {}


def _build_nc():
    nc = bacc.Bacc()
    vbt = nc.dram_tensor("vbt", [D, NPC], _BF16, kind="ExternalInput")
    m1 = nc.dram_tensor("m1", [P, TILES, A], _F32, kind="ExternalInput")
    m2 = nc.dram_tensor("m2", [P, TILES, A], _F32, kind="ExternalInput")
    wct = nc.dram_tensor("wct", [D, F], _BF16, kind="ExternalInput")
    ind = nc.dram_tensor("ind", [P, 2], _F32R, kind="ExternalInput")
    indt = nc.dram_tensor("indt", [2, P], _F32R, kind="ExternalInput")
    o_pna = nc.dram_tensor("pna", [NPC, A], _F32, kind="ExternalOutput")
    o_pa = nc.dram_tensor("pa", [GPC, A], _F32, kind="ExternalOutput")
    o_t = nc.dram_tensor("tt", [GPC, A], _F32, kind="ExternalOutput")
    o_h = nc.dram_tensor("hh", [GPC, A], _F32, kind="ExternalOutput")

    pna_v = o_pna.rearrange("(b t p) a -> b p t a", b=NB, t=TPB, p=P)
    pa_v = o_pa.rearrange("(b t g) a -> g b t a", b=NB, t=TPB, g=2)
    t_v = o_t.rearrange("(b t g) a -> g b t a", b=NB, t=TPB, g=2)
    h_v = o_h.rearrange("(b t g) a -> g b t a", b=NB, t=TPB, g=2)

    AF = mybir.ActivationFunctionType

    with nc.allow_low_precision(reason="float32r matmul rhs; ~19-bit mantissa ample for 2e-2 tol"), TileContext(nc) as tc:
        with (
            tc.tile_pool(name="const", bufs=1) as cpool,
            tc.tile_pool(name="io", bufs=3) as iop,
            tc.tile_pool(name="work", bufs=2) as wp,
            tc.tile_pool(name="psl", bufs=2, space="PSUM") as psl,
            tc.tile_pool(name="pss", bufs=2, space="PSUM") as pss,
            tc.tile_pool(name="psb", bufs=2, space="PSUM") as psb,
            tc.tile_pool(name="psa", bufs=2, space="PSUM") as psa,
        ):
            wca = cpool.tile([P, F], _BF16, tag="wca")
            nc.sync.dma_start(out=wca[:, :], in_=wct[0:P, :])
            wcb = cpool.tile([P, F], _BF16, tag="wcb")
            nc.sync.dma_start(out=wcb[:, :], in_=wct[P:D, :])
            ind_t = cpool.tile([P, 2], _F32R, tag="ind")
            nc.sync.dma_start(out=ind_t[:, :], in_=ind[:, :])
            indt_t = cpool.tile([2, P], _F32R, tag="indt")
            nc.sync.dma_start(out=indt_t[:, :], in_=indt[:, :])
            epsb = cpool.tile([P, 1], _F32, tag="eps")
            nc.vector.memset(epsb[:, :], EPS)
            s_pa = cpool.tile([2, NB, TPB, A], _F32, tag="spa")
            s_t = cpool.tile([2, NB, TPB, A], _F32, tag="st")
            s_h = cpool.tile([2, NB, TPB, A], _F32, tag="sh")

            wc0 = wca[:, :]
            wc1 = wcb[:, :]
            ind_r = ind_t[:, :]
            indt_r = indt_t[:, :]

            for b in range(NB):
                vb0 = iop.tile([P, TPB * P], _BF16, tag="vb0")
                nc.sync.dma_start(out=vb0[:, :], in_=vbt[0:P, b * TPB * P:(b + 1) * TPB * P])
                vb1 = iop.tile([P, TPB * P], _BF16, tag="vb1")
                nc.sync.dma_start(out=vb1[:, :], in_=vbt[P:D, b * TPB * P:(b + 1) * TPB * P])
                m1t = iop.tile([P, TPB, A], _F32, tag="m1")
                nc.sync.dma_start(out=m1t[:, :, :], in_=m1[:, b * TPB:(b + 1) * TPB, :])
                m2t = iop.tile([P, TPB, A], _F32, tag="m2")
                nc.sync.dma_start(out=m2t[:, :, :], in_=m2[:, b * TPB:(b + 1) * TPB, :])

                z = wp.tile([P, TPB, 65], _F32, tag="z")
                q = wp.tile([P, TPB, A], _F32, tag="q")
                for t in range(TPB):
                    lp = psl.tile([P, F], _F32, tag="l")
                    nc.tensor.matmul(lp[:, :], vb0[:, t * P:(t + 1) * P], wc0,
                                     start=True, stop=False)
                    nc.tensor.matmul(lp[:, :], vb1[:, t * P:(t + 1) * P], wc1,
                                     start=False, stop=True)
                    nc.scalar.activation(z[:, t, :].bitcast(_F32R), lp[:, 0:65], AF.Exp)
                    nc.scalar.activation(q[:, t, :], lp[:, 65:F], AF.Copy)

                za = wp.tile([P, TPB, A], _F32, tag="za")
                nc.vector.tensor_mul(za[:, :, :], z[:, :, 1:33], m1t[:, :, :])
                zn = wp.tile([P, TPB, A], _F32, tag="zn")
                nc.vector.tensor_mul(zn[:, :, :].bitcast(_F32R), z[:, :, 33:65], m2t[:, :, :])
                sa = wp.tile([P, TPB], _F32, tag="sa")
                nc.vector.reduce_sum(sa[:, :], za[:, :, :], axis=mybir.AxisListType.X)

                s_all = pss.tile([2, TPB, 34], _F32, tag="s")
                nc.tensor.matmul(s_all[:, :, 2:34], ind_r, zn[:, :, :].bitcast(_F32R),
                                 start=True, stop=True)
                nc.tensor.matmul(s_all[:, :, 0:2], ind_r, z[:, :, 0:2].bitcast(_F32R),
                                 start=True, stop=True)
                r_all = wp.tile([2, TPB, 34], _F32, tag="r")
                nc.vector.reciprocal(r_all[:, :, :].bitcast(_F32R), s_all[:, :, :])

                b_all = psb.tile([P, TPB, 34], _F32, tag="b")
                nc.tensor.matmul(b_all[:, :, :], indt_r, r_all[:, :, :].bitcast(_F32R),
                                 start=True, stop=True)

                pn = wp.tile([P, TPB], _F32, tag="pn")
                nc.vector.tensor_mul(pn[:, :], z[:, :, 0], b_all[:, :, 0])
                rsa = wp.tile([P, TPB], _F32, tag="rsa")
                nc.vector.reciprocal(rsa[:, :], sa[:, :])
                w = wp.tile([P, TPB], _F32, tag="w")
                nc.vector.tensor_mul(w[:, :], pn[:, :], rsa[:, :])

                pna = wp.tile([P, TPB, A], _F32, tag="pna")
                nc.vector.tensor_mul(pna[:, :, :], zn[:, :, :], b_all[:, :, 2:34])
                # contrib = za * w  (w broadcast over the action axis, step 0)
                w_ap = w[:, :]
                w_b = bass.AP(tensor=w_ap.tensor, offset=w_ap.offset,
                              ap=[list(p) for p in w_ap.ap] + [[0, A]])
                contrib = wp.tile([P, TPB, A], _F32, tag="contrib")
                nc.vector.tensor_mul(contrib[:, :, :].bitcast(_F32R), za[:, :, :], w_b)

                logp = wp.tile([P, TPB, A], _F32, tag="logp")
                nc.scalar.activation(logp[:, :, :], pna[:, :, :], AF.Ln, bias=epsb[:, 0:1])
                plog = wp.tile([P, TPB, A], _F32, tag="plog")
                nc.vector.tensor_mul(plog[:, :, :].bitcast(_F32R), pna[:, :, :], logp[:, :, :])
                tq = wp.tile([P, TPB, A], _F32, tag="tq")
                nc.vector.tensor_mul(tq[:, :, :].bitcast(_F32R), pna[:, :, :], q[:, :, :])

                pa_ps = psa.tile([2, TPB, A], _F32, tag="acc")
                nc.tensor.matmul(pa_ps[:, :, :], ind_r, contrib[:, :, :].bitcast(_F32R),
                                 start=True, stop=True)
                t_ps = psa.tile([2, TPB, A], _F32, tag="acc")
                nc.tensor.matmul(t_ps[:, :, :], ind_r, tq[:, :, :].bitcast(_F32R),
                                 start=True, stop=True)
                h_ps = psa.tile([2, TPB, A], _F32, tag="acc")
                nc.tensor.matmul(h_ps[:, :, :], ind_r, plog[:, :, :].bitcast(_F32R),
                                 start=True, stop=True)
                nc.scalar.activation(s_pa[:, b, :, :], pa_ps[:, :, :], AF.Copy)
                nc.vector.tensor_copy(s_t[:, b, :, :], t_ps[:, :, :])
                nc.vector.tensor_copy(s_h[:, b, :, :], h_ps[:, :, :])

                nc.sync.dma_start(out=pna_v[b, :, :, :], in_=pna[:, :, :])

            nc.sync.dma_start(out=pa_v[:, :, :, :], in_=s_pa[:, :, :, :])
            nc.sync.dma_start(out=t_v[:, :, :, :], in_=s_t[:, :, :, :])
            nc.sync.dma_start(out=h_v[:, :, :, :], in_=s_h[:, :, :, :])
    nc.finalize()
    return nc


def _prep(values, action_type_mask, action_arity_mask, W_node, W_agn, W_nga, W_q):
    vb = values.astype(ml_dtypes.bfloat16)
    vbt = np.ascontiguousarray(vb.T)  # [D, N]
    m1f = action_type_mask.astype(np.float32)
    m12f = np.logical_and(action_type_mask, action_arity_mask).astype(np.float32)

    def pack(m):  # [N, A] -> per-core [P, TILES, A]
        r = m.reshape(CORES, TILES, P, A)
        return [np.ascontiguousarray(r[c].transpose(1, 0, 2)) for c in range(CORES)]

    m1p, m2p = pack(m1f), pack(m12f)
    wq_eff = 0.5 * (W_q[0:A] + W_q[A:2 * A])
    wcat = np.concatenate([W_node, W_agn, W_nga, wq_eff], axis=0)  # [97, 256]
    wct = np.ascontiguousarray(wcat.T).astype(ml_dtypes.bfloat16)  # [256, 97]
    ind = np.zeros((P, 2), np.float32)
    ind[0:NPG, 0] = 1.0
    ind[NPG:P, 1] = 1.0
    indt = np.ascontiguousarray(ind.T)
    in_maps = []
    for c in range(CORES):
        in_maps.append({
            "vbt": np.ascontiguousarray(vbt[:, c * NPC:(c + 1) * NPC]),
            "m1": m1p[c], "m2": m2p[c], "wct": wct, "ind": ind, "indt": indt,
        })
    return in_maps


def _numpy_fallback(a, values, indices, action_type_mask, action_arity_mask,
                    n_nodes, W_node, W_agn, W_nga, W_q):
    n_g = n_nodes.shape[0]
    seg = indices.astype(np.int64)

    def segsum(x):
        out = np.zeros((n_g,) + x.shape[1:], x.dtype)
        np.add.at(out, seg, x)
        return out

    node_logits = (values @ W_node.T).squeeze(-1)
    agn = values @ W_agn.T
    nga = values @ W_nga.T
    m1 = action_type_mask
    m2 = np.logical_and(action_arity_mask, action_type_mask)
    zn = np.exp(node_logits)
    p_n = zn / segsum(zn)[seg]
    za = np.exp(agn) * m1
    p_agn = za / za.sum(-1, keepdims=True)
    p_a = segsum(p_n[:, None] * p_agn)
    zg = np.exp(nga) * m2
    p_n__a = zg / segsum(zg)[seg]
    q = (values @ W_q.T).reshape(-1, HEADS, A).mean(axis=1)
    value = np.sum(p_a * segsum(p_n__a * q), axis=-1)
    a_act, a_node = a[:, 0].astype(np.int64), a[:, 1].astype(np.int64)
    g = np.arange(n_g)
    logprob = np.log(p_a[g, a_act] + EPS) + np.log(p_n__a[a_node, a_act] + EPS)
    ent_a = -np.sum(p_a * np.log(p_a + EPS), axis=-1)
    h_n = -segsum(p_n__a * np.log(p_n__a + EPS))
    entropy = ent_a + np.sum(p_a * h_n, axis=-1)
    return (logprob.astype(np.float32), entropy.astype(np.float32),
            value.astype(np.float32), p_a.astype(np.float32),
            p_n__a.astype(np.float32))


def kernel(a, values, indices, action_type_mask, action_arity_mask, n_nodes,
           W_node, W_agn, W_nga, W_q):
    a = np.asarray(a)
    values = np.asarray(values, dtype=np.float32)
    indices = np.asarray(indices)
    action_type_mask = np.asarray(action_type_mask)
    action_arity_mask = np.asarray(action_arity_mask)
    n_nodes = np.asarray(n_nodes)
    W_node = np.asarray(W_node, dtype=np.float32)
    W_agn = np.asarray(W_agn, dtype=np.float32)
    W_nga = np.asarray(W_nga, dtype=np.float32)
    W_q = np.asarray(W_q, dtype=np.float32)

    balanced = (values.shape == (N, D)
                and np.array_equal(indices.astype(np.int64),
                                   np.arange(N, dtype=np.int64) // NPG))
    if not balanced:
        return _numpy_fallback(a, values, indices, action_type_mask,
                               action_arity_mask, n_nodes, W_node, W_agn,
                               W_nga, W_q)

    if "nc" not in _CACHE:
        _CACHE["nc"] = _build_nc()
    nc = _CACHE["nc"]
    in_maps = _prep(values, action_type_mask, action_arity_mask,
                    W_node, W_agn, W_nga, W_q)
    res = run_bass_kernel_spmd(nc, in_maps, core_ids=list(range(CORES)))
    _CACHE["last_exec_time_ns"] = res.exec_time_ns
    _CACHE["last_trace_info"] = (res.mean_exec_time_ns, res.max_exec_time_core_id)

    p_n__a = np.concatenate([res.results[c]["pna"] for c in range(CORES)], axis=0)
    p_a = np.concatenate([res.results[c]["pa"] for c in range(CORES)], axis=0)
    t = np.concatenate([res.results[c]["tt"] for c in range(CORES)], axis=0)
    h = np.concatenate([res.results[c]["hh"] for c in range(CORES)], axis=0)

    h_n = -h
    value = np.sum(p_a * t, axis=-1)
    ent_a = -np.sum(p_a * np.log(p_a + EPS), axis=-1)
    entropy = ent_a + np.sum(p_a * h_n, axis=-1)
    a_act = a[:, 0].astype(np.int64)
    a_node = a[:, 1].astype(np.int64)
    g = np.arange(N_GRAPHS)
    logprob = (np.log(p_a[g, a_act] + EPS)
               + np.log(p_n__a[a_node, a_act] + EPS))
    return (logprob.astype(np.float32), entropy.astype(np.float32),
            value.astype(np.float32), p_a.astype(np.float32),
            p_n__a.astype(np.float32))
